# revision 19
# baseline (speedup 1.0000x reference)
"""BinaryLinear kernel for 8x TRN2 NeuronCores.

out = x @ (weight > 0)  with x [8192, 2048] f32, weight [2048, 2048] f32.

Sharding: data-parallel over batch (1024 rows/core), weight replicated.

Per core (M=1024, K=2048, N=2048). The kernel is DMA-wire-bound: 32MiB
of DMA per core (8 x + 16 w in, 8 out) = 93.2us at the 360GB/s per-core
DMA rate. v2 pushes the PE work far enough below that floor that the
schedule can hug it.

- Split-K mixed precision with MEAN-CORRECTION: k-tiles 0..1 run in
  bf16; k-tiles 2..15 run in fp8e4 DoubleRow (2 k-tiles per matmul,
  2x PE rate; binarized {0,1} weights are exact in fp8, x is fp8e4-
  quantized). The fp8 quantization error e_k = x8-x enters the output
  as sum_k e_k w_kn; writing w = 0.5 + (w-0.5), the 0.5*sum(e_k) term
  (half the error variance) is removed exactly:
    * S8[m] = sum_fp8k x8[k,m] accumulates via tiny piggyback DoubleRow
      matmuls (rhs = a [P,2,1] const-0.5 column) that share the
      stationary xT8 operand with the main matmuls (LDWEIGHTS dedup
      makes them ~free);
    * Sx[m] = exact f32 row-sum of x over the fp8 k-range via DVE
      tensor_reduce on the raw x tiles;
    * c[m] = 0.5*(Sx - S8) is folded into every PSUM eviction as a
      per-partition tensor_scalar add on DVE.
  Net rel err 1.76e-2 (numpy-modeled on the reference inputs,
  verified on HW), under the 2e-2 gate; PE matmul floor drops from
  89us (6 fp8 kts uncorrected) to 61us.
- Weight streams in three column tranches (1024 + 512 + 512 cols),
  k-tile-major within each; every tranche-0 k-tile streams as two
  256KB nt-half DMAs so each arriving half immediately unlocks wave
  matmuls. DVE binarizes to {0,1} (bf16 or fp8) per 512-col quarter.
- Ramp: while tranche-0 k-tiles arrive, matmuls run kt/ktp-OUTER
  across 6 live banks (bt0,bt1,bt2)x(nt0,nt1) with bt2 joining at
  DR-pair 3 and wrapping its missed k-tiles afterwards; x transposes
  fill PE slack.
- Steady state: nt-paired 2-bank groups, kt-inner, consecutive
  matmuls sharing the stationary xT tile; each (0,1) pair carries the
  next bt's cast(DVE)/transpose(PE)/evict(ACT) chain interleaved.
- Late phase: the (nt2, nt3) work is gated on the last 23us of the
  input stream, so it runs kt-OUTER as a second wave across 6 bts
  (tranche-2 then tranche-3), with the remaining bts' groups kt-inner
  at the end; the last bt splits into half-width accumulations so
  eviction overlaps the final matmuls.
- All out-DMAs ride the SP HWDGE ring EMITTED AFTER every input DMA:
  ring FIFO order gives inputs strict wire priority; a deep SBUF
  staging pool absorbs evictions until the input stream drains.
- x transposed 128x128-blockwise on the PE (is_transpose), 4 blocks
  per PSUM staging tile, contiguous ACT eviction into per-bt xT/xT8
  tiles; the first two startup chunks transpose raw f32 directly.
"""

import numpy as np

import concourse.bass as bass
import concourse.mybir as mybir
import concourse.tile as tile
from concourse import bacc
from concourse.bass_utils import run_bass_kernel_spmd
from concourse.masks import make_identity

B, K, N = 8192, 2048, 2048
N_CORES = 8
MB = B // N_CORES          # 1024 batch rows per core
P = 128
KT = K // P                # 16 k-tiles
BT = MB // P               # 8 batch tiles per core
NT = 4                     # output column blocks of 512
NB = N // NT               # 512
HW = K // 2                # 1024

F32 = mybir.dt.float32
BF16 = mybir.dt.bfloat16
F8 = mybir.dt.float8e4

# k-tiles >= KT_F8 run in fp8e4 DoubleRow; mean-corrected (see module
# docstring). KT_F8=2 -> 14 fp8 k-tiles, 7 DR pairs.
KT_F8 = 4
KTP0 = KT_F8 // 2          # first DR pair index
NKTP = KT // 2 - KTP0      # number of DR pairs
JOIN_KTP = KTP0 + 2        # DR pair at which bt2 joins the ramp wave


def build_kernel(repeat: int = 1, mode: str = "full"):
    nc = bacc.Bacc(None, target_bir_lowering=False)
    x = nc.dram_tensor("x", [MB, K], F32, kind="ExternalInput")
    w = nc.dram_tensor("w", [K, N], F32, kind="ExternalInput")
    out = nc.dram_tensor("out", [MB, N], F32, kind="ExternalOutput")

    w3 = w[:].rearrange("(kt p) n -> p kt n", p=P)   # [128, 16, 2048]

    do_x = mode in ("full", "nomm", "xonly")
    do_w = mode in ("full", "nomm", "wonly")
    do_mm = mode in ("full", "mmonly")

    def body(tc, pools):
        (xraw_pool, xbf_pool, xT_pool, wraw_pool, wbin_pool,
         out_pool, psum_pool, tpsum_pool, pss_pool, corr_pool,
         const_pool) = pools

        ident = const_pool.tile([P, P], BF16, tag="ident", name="ident")
        ident32 = const_pool.tile([P, P], F32, tag="ident32", name="ident32")
        halfcol = const_pool.tile([P, 2], F8, tag="halfcol", name="halfcol")
        # one persistent PSUM bank holds every bt's 0.5*S8 accumulator
        # (column bt). A matmul with start=True zeroes the whole bank
        # (not just its own column), so the bank is memset once and every
        # piggyback matmul accumulates with start=False.
        pss_all = pss_pool.tile([P, BT], F32, tag="pss", name="pss")

        def const_setup():
            make_identity(nc, ident)
            make_identity(nc, ident32)
            nc.any.memset(halfcol[:], 0.5)
            nc.any.memset(pss_all[:], 0.0)

        xraw = {}   # (bt, half) -> [P, HW] f32
        xT = {}     # bt -> [P, KT_F8*P] bf16
        xT8 = {}    # bt -> [P, (KT-KT_F8)*P] fp8e4 (col = (kt-KT_F8)*P + m)
        wbin = {}   # (kt, nt) -> [P, NB] bf16      (kts < KT_F8)
        wbin8 = {}  # (ktp, nt) -> [P, 2*NB] fp8e4  (ktp in KTP0..KT//2-1)
        sxp = {}    # (bt, half) -> [P, 1] f32 partial exact row-sums
        pss = {}    # bt -> [P, 1] f32 psum tile accumulating 0.5*S8
        cbias = {}  # bt -> [P, 1] f32 sbuf correction tile
        evict_flip = [0]

        def _binarize(dst, src):
            nc.vector.tensor_scalar(out=dst, in0=src, scalar1=0.0,
                                    scalar2=None, op0=mybir.AluOpType.is_gt)

        def _wbin_dst(kt, nt):
            """Destination slice for a binarized [P, NB] w quarter."""
            if kt < KT_F8:
                wbin[kt, nt] = wbin_pool.tile(
                    [P, NB], BF16, tag=f"wbin_{kt}_{nt}", name=f"wb{kt}_{nt}")
                return wbin[kt, nt][:]
            ktp, j = divmod(kt, 2)
            if (ktp, nt) not in wbin8:
                wbin8[ktp, nt] = wbin_pool.tile(
                    [P, 2 * NB], F8, tag=f"wbin8_{ktp}_{nt}",
                    name=f"wb8{ktp}_{nt}")
            return wbin8[ktp, nt][:, j * NB:(j + 1) * NB]

        def dma_x_piece(bt, half, piece):
            if not do_x:
                return
            if piece == 0:
                xraw[bt, half] = xraw_pool.tile(
                    [P, HW], F32, tag=f"xraw_{half}", name=f"xr{bt}_{half}")
            t = xraw[bt, half]
            h0 = half * HW
            c = piece * NB
            nc.sync.dma_start(t[:, c:c + NB],
                              x[bt * P:(bt + 1) * P, h0 + c:h0 + c + NB])

        def dma_x(bt, half, quarters=False):
            if not do_x:
                return
            t = xraw_pool.tile([P, HW], F32, tag=f"xraw_{half}",
                               name=f"xr{bt}_{half}")
            h0 = half * HW
            if quarters:
                nc.sync.dma_start(t[:, :NB], x[bt * P:(bt + 1) * P,
                                               h0:h0 + NB])
                nc.sync.dma_start(t[:, NB:], x[bt * P:(bt + 1) * P,
                                               h0 + NB:h0 + HW])
            else:
                nc.sync.dma_start(t[:], x[bt * P:(bt + 1) * P, h0:h0 + HW])
            xraw[bt, half] = t

        def dma_w2(kt):
            """Tranche-0: [128,1024] w k-tile -> binarized quarters nt0,nt1."""
            dsts = [_wbin_dst(kt, j) for j in range(2)]
            if do_w:
                wr = wraw_pool.tile([P, HW], F32, tag="wraw2", name="wr")
                nc.sync.dma_start(wr[:], w3[:, kt, 0:HW])
                for j in range(2):
                    _binarize(dsts[j], wr[:, j * NB:(j + 1) * NB])
            else:
                for j in range(2):
                    nc.any.memset(dsts[j], 1.0)

        def dma_w1(kt, nt):
            """Tranche-2/3: [128,512] w k-tile quarter."""
            dst = _wbin_dst(kt, nt)
            if do_w:
                wr = wraw_pool.tile([P, NB], F32, tag="wraw1", name="wr")
                nc.sync.dma_start(wr[:], w3[:, kt, nt * NB:(nt + 1) * NB])
                _binarize(dst, wr[:])
            else:
                nc.any.memset(dst, 1.0)

        # ---- exact f32 row-sums of x over the fp8 k-range (DVE) ----
        def sx_reduce(bt, half):
            if not (do_x and do_mm):
                return
            lo = max(KT_F8 * P - half * HW, 0)
            t = corr_pool.tile([P, 1], F32, tag=f"sxp_{half}",
                               name=f"sx{bt}_{half}")
            nc.vector.tensor_reduce(
                t[:], xraw[bt, half][:, lo:HW],
                axis=mybir.AxisListType.X, op=mybir.AluOpType.add)
            sxp[bt, half] = t

        def compute_c(bt):
            """c[bt] = 0.5*(Sx - S8); the per-partition eviction bias."""
            if not do_mm:
                return
            sx = corr_pool.tile([P, 1], F32, tag="sx", name=f"sxc{bt}")
            if (bt, 0) in sxp:
                # sx = 0.5 * (sx_half0 + sx_half1)
                nc.vector.tensor_scalar(
                    out=sx[:], in0=sxp.pop((bt, 0))[:],
                    scalar1=sxp.pop((bt, 1))[:], scalar2=0.5,
                    op0=mybir.AluOpType.add, op1=mybir.AluOpType.mult)
            else:
                nc.any.memset(sx[:], 0.0)
            c = corr_pool.tile([P, 1], F32, tag="c", name=f"c{bt}")
            # pss holds 0.5*S8; c = 0.5*Sx - 0.5*S8 = (pss - sx) * -1
            nc.vector.tensor_scalar(
                out=c[:], in0=pss[bt], scalar1=sx[:], scalar2=-1.0,
                op0=mybir.AluOpType.subtract, op1=mybir.AluOpType.mult)
            cbias[bt] = c

        xbf = {}

        def cast_chunk(bt, ktg, on_dve=False):
            """Cast 512 cols of x(bt) f32 -> bf16 (ACT, or DVE for the
            steady-phase bts)."""
            if bt not in xT:
                xT[bt] = xT_pool.tile([P, KT_F8 * P], BF16, tag=f"xT_{bt}",
                                      name=f"xT_{bt}")
                xT8[bt] = xT_pool.tile([P, (KT - KT_F8) * P], F8,
                                       tag=f"xT8_{bt}", name=f"xT8_{bt}")
            if not do_x:
                if ktg == 0:
                    nc.any.memset(xT[bt][:], 1.0)
                    nc.any.memset(xT8[bt][:], 1.0)
                return
            half, off = divmod(ktg * 4 * P, HW)
            xb = xbf_pool.tile([P, 4 * P], BF16, tag=f"xbf_{ktg % 2}",
                               name=f"xbf{bt}_{ktg}")
            src = xraw[bt, half][:, off:off + 4 * P]
            if on_dve:
                nc.vector.tensor_copy(xb[:], src)
            else:
                nc.scalar.activation(xb[:], src,
                                     mybir.ActivationFunctionType.Copy)
            xbf[bt, ktg] = xb

        def _evict_T(dst_seg_f32, tp, bt, ktg):
            """Segmented eviction of a 4-block transposed chunk into
            xT (bf16 kts) and xT8 (fp8 kts)."""
            k0 = ktg * 4
            nbf = min(max(KT_F8 - k0, 0), 4)
            if nbf:
                nc.scalar.activation(
                    xT[bt][:, k0 * P:(k0 + nbf) * P], tp[:, :nbf * P],
                    mybir.ActivationFunctionType.Copy)
            if nbf < 4:
                f0 = (k0 + nbf - KT_F8) * P
                nc.scalar.activation(
                    xT8[bt][:, f0:f0 + (4 - nbf) * P], tp[:, nbf * P:],
                    mybir.ActivationFunctionType.Copy)

        def transp_f32_chunk(bt, ktg):
            """Startup only: transpose 4 blocks straight from raw f32 x
            (skips the cast on the critical path); PSUM staging borrows a
            main-pool bank; ACT eviction converts f32 -> bf16/fp8."""
            if bt not in xT:
                xT[bt] = xT_pool.tile([P, KT_F8 * P], BF16, tag=f"xT_{bt}",
                                      name=f"xT_{bt}")
                xT8[bt] = xT_pool.tile([P, (KT - KT_F8) * P], F8,
                                       tag=f"xT8_{bt}", name=f"xT8_{bt}")
            if not do_x:
                if ktg == 0:
                    nc.any.memset(xT[bt][:], 1.0)
                    nc.any.memset(xT8[bt][:], 1.0)
                return
            half, off = divmod(ktg * 4 * P, HW)
            tp = psum_pool.tile([P, 4 * P], F32, tag="ps", name="tpf")
            for i in range(4):
                nc.tensor.transpose(
                    tp[:, i * P:(i + 1) * P],
                    xraw[bt, half][:, off + i * P:off + (i + 1) * P],
                    ident32[:])
            _evict_T(None, tp, bt, ktg)

        def transp_chunk(bt, ktg):
            """PE: transpose 4 blocks into one PSUM staging tile; ACT
            evicts contiguously into xT[bt] / xT8[bt]."""
            if not do_x:
                return
            xb = xbf.pop((bt, ktg))
            tp = tpsum_pool.tile([P, 4 * P], BF16, tag="tps", name="tp")
            for i in range(4):
                nc.tensor.transpose(tp[:, i * P:(i + 1) * P],
                                    xb[:, i * P:(i + 1) * P], ident[:])
            _evict_T(None, tp, bt, ktg)

        def cast_T(bt, ktg):
            cast_chunk(bt, ktg)
            transp_chunk(bt, ktg)

        def mm(ps, bt, nt, kt, start, stop):
            nc.tensor.matmul(
                ps[:], xT[bt][:, kt * P:(kt + 1) * P], wbin[kt, nt][:],
                start=start, stop=stop)

        def mm_dr(ps, bt, nt, ktp, stop, start=False):
            """fp8 DoubleRow matmul: contracts k-tiles 2*ktp and 2*ktp+1
            in one pass (2 fp8 weights per PE cell, 2x rate)."""
            a = (ktp - KTP0) * 2 * P
            lhsT = xT8[bt][:, a:a + 2 * P].rearrange("p (j m) -> p j m", j=2)
            rhs = wbin8[ktp, nt][:].rearrange("p (j n) -> p j n", j=2)
            nc.tensor.matmul(ps[:], lhsT, rhs, start=start, stop=stop,
                             perf_mode=mybir.MatmulPerfMode.DoubleRow)

        def mm_s8(bt, ktp, start, stop):
            """Piggyback DR matmul accumulating 0.5*S8[m] into column bt
            of the shared pss bank. Shares the stationary xT8 pair with
            the main matmul emitted just before it, so its LDWEIGHTS
            dedups away."""
            if not do_mm:
                return
            pss[bt] = pss_all[:, bt:bt + 1]
            a = (ktp - KTP0) * 2 * P
            lhsT = xT8[bt][:, a:a + 2 * P].rearrange("p (j m) -> p j m", j=2)
            rhs = halfcol[:].rearrange("p (j n) -> p j n", j=2)
            nc.tensor.matmul(pss[bt], lhsT, rhs, start=False, stop=stop,
                             perf_mode=mybir.MatmulPerfMode.DoubleRow,
                             skip_group_check=True)

        def evict_out(ps, bt, nt, off=0, ring=None):
            """PSUM -> SBUF staging with the mean-correction bias folded
            in (DVE tensor_scalar add of c[bt]) -> DRAM via the SP HWDGE
            ring (emitted after every input DMA: FIFO order gives inputs
            strict wire priority). Evicts the whole ps tile to out cols
            [nt*NB + off, ...)."""
            ring = ring or nc.sync
            wd = ps.shape[1]
            ot = out_pool.tile([P, wd], F32, tag="osb", name="ot")
            if do_mm:
                nc.vector.tensor_scalar(
                    out=ot[:], in0=ps[:],
                    scalar1=cbias[bt][:], scalar2=None,
                    op0=mybir.AluOpType.add)
            else:
                nc.vector.tensor_copy(ot[:], ps[:])
            ring.dma_start(
                out[bt * P:(bt + 1) * P,
                    nt * NB + off:nt * NB + off + wd], ot[:])

        def group(bt, nt, ring=None, split=1):
            """Single-nt group; split=2 runs two half-width (256-col)
            accumulations so the first half's eviction + out-DMA overlap
            the second half's matmuls (shrinks the kernel tail)."""
            ring = ring or nc.sync
            wd = NB // split
            for h in range(split):
                ps = psum_pool.tile([P, wd], F32, tag="ps", name="ps")
                if do_mm:
                    for kt in range(KT_F8):
                        nc.tensor.matmul(
                            ps[:], xT[bt][:, kt * P:(kt + 1) * P],
                            wbin[kt, nt][:, h * wd:(h + 1) * wd],
                            start=(kt == 0), stop=False)
                    for ktp in range(KTP0, KT // 2):
                        a = (ktp - KTP0) * 2 * P
                        lhsT = xT8[bt][:, a:a + 2 * P].rearrange(
                            "p (j m) -> p j m", j=2)
                        rhs = wbin8[ktp, nt][:].rearrange(
                            "p (j n) -> p j n", j=2)[:, :, h * wd:(h + 1) * wd]
                        nc.tensor.matmul(
                            ps[:], lhsT, rhs, start=False,
                            stop=(ktp == KT // 2 - 1),
                            perf_mode=mybir.MatmulPerfMode.DoubleRow)
                else:
                    nc.any.memset(ps[:], 0.0)
                evict_out(ps, bt, nt, off=h * wd, ring=ring)

        def group_pair(bt, nts, ring=None, prep=None):
            """Two nt blocks of one bt, kt-inner with consecutive matmuls
            sharing the stationary xT tile. The (nt0, nt1) pair carries
            the bt's S8 piggyback chain; `prep` names a bt whose
            cast/transpose chain is emitted interleaved with the DR
            pairs."""
            s8_here = nts[0] == 0
            psa = psum_pool.tile([P, NB], F32, tag="ps", name="ps")
            psb = psum_pool.tile([P, NB], F32, tag="ps", name="ps")
            prep_at = {2: 0, 3: 1, 4: 2, 5: 3}
            if do_mm:
                for kt in range(KT_F8):
                    mm(psa, bt, nts[0], kt, kt == 0, False)
                    mm(psb, bt, nts[1], kt, kt == 0, False)
                for ktp in range(KTP0, KT // 2):
                    last = ktp == KT // 2 - 1
                    mm_dr(psa, bt, nts[0], ktp, last)
                    if s8_here:
                        mm_s8(bt, ktp, ktp == KTP0, last)
                    mm_dr(psb, bt, nts[1], ktp, last)
                    if prep is not None and ktp in prep_at:
                        g = prep_at[ktp]
                        cast_chunk(prep, g, on_dve=True)
                        transp_chunk(prep, g)
                        if g == 3:
                            sx_reduce(prep, 0)
                            sx_reduce(prep, 1)
            else:
                nc.any.memset(psa[:], 0.0)
                nc.any.memset(psb[:], 0.0)
                if prep is not None:
                    for g in range(4):
                        cast_T(prep, g)
            if s8_here:
                compute_c(bt)
            evict_out(psa, bt, nts[0], ring=ring)
            evict_out(psb, bt, nts[1], ring=ring)

        # ================= emission script =================
        # startup: first x chunks and w k-tiles interleaved so the first
        # wave matmul can issue as early as possible
        dma_x(0, 0, quarters=True)
        dma_w1(0, 0)
        dma_w1(0, 1)
        dma_x(0, 1)
        const_setup()
        dma_w1(1, 0)
        dma_x_piece(1, 0, 0)
        dma_w1(1, 1)
        dma_x_piece(1, 0, 1)
        dma_w1(2, 0)
        dma_x_piece(1, 1, 0)
        dma_w1(2, 1)
        dma_x_piece(1, 1, 1)
        dma_w1(3, 0)
        dma_x_piece(2, 0, 0)
        dma_w1(3, 1)
        dma_x_piece(2, 0, 1)
        dma_w1(4, 0)
        dma_x_piece(2, 1, 0)
        dma_w1(4, 1)
        dma_x_piece(2, 1, 1)
        for kt in range(5, KT):
            dma_w1(kt, 0)
            dma_w1(kt, 1)
            if kt == 7:
                dma_x(3, 0)
        dma_x(3, 1)
        # rest of x right after tranche-0; then tranches 2 and 3
        for bt in range(4, BT):
            dma_x(bt, 0)
            dma_x(bt, 1)
        for kt in range(KT):
            dma_w1(kt, 2)
        for kt in range(KT):
            dma_w1(kt, 3)

        # transposes for bt0, bt1 up front (bt0's first two chunks
        # transpose raw f32 directly - no cast latency on the critical
        # startup path)
        transp_f32_chunk(0, 0)
        transp_f32_chunk(0, 1)
        cast_T(0, 2)
        cast_T(0, 3)
        for ktg in range(4):
            cast_T(1, ktg)
        sx_reduce(0, 0)
        sx_reduce(0, 1)
        sx_reduce(1, 0)
        sx_reduce(1, 1)

        # ramp wave: ktp-outer, (bt0,bt1)x(nt0,nt1); bt2 joins at
        # JOIN_KTP and wraps around; transposes for bt2 fill PE slack,
        # bt3's h0 casts are prefetched near the wave end
        if do_mm:
            wave = [(0, 0), (0, 1), (1, 0), (1, 1)]
            wps = {g: psum_pool.tile([P, NB], F32, tag="ps", name=f"wps{g}")
                   for g in wave}
            join = [(2, 0), (2, 1)]
            for kt in range(KT_F8):
                for bt_, nt_ in wave:
                    mm(wps[bt_, nt_], bt_, nt_, kt, kt == 0, False)
            for ktp in range(KTP0, KT // 2):
                last = ktp == KT // 2 - 1
                mm_dr(wps[0, 0], 0, 0, ktp, last)
                mm_s8(0, ktp, ktp == KTP0, last)
                mm_dr(wps[0, 1], 0, 1, ktp, last)
                mm_dr(wps[1, 0], 1, 0, ktp, last)
                mm_s8(1, ktp, ktp == KTP0, last)
                mm_dr(wps[1, 1], 1, 1, ktp, last)
                if ktp == JOIN_KTP - 2:
                    cast_T(2, 0)
                    cast_T(2, 1)
                elif ktp == JOIN_KTP - 1:
                    cast_T(2, 2)
                    cast_T(2, 3)
                    sx_reduce(2, 0)
                    sx_reduce(2, 1)
                    for g in join:
                        wps[g] = psum_pool.tile([P, NB], F32, tag="ps",
                                                name=f"wps{g}")
                if ktp >= JOIN_KTP:
                    mm_dr(wps[2, 0], 2, 0, ktp, False, start=ktp == JOIN_KTP)
                    mm_s8(2, ktp, ktp == JOIN_KTP, False)
                    mm_dr(wps[2, 1], 2, 1, ktp, False)
                if ktp == KT // 2 - 2:
                    cast_chunk(3, 0)
                elif ktp == KT // 2 - 1:
                    cast_chunk(3, 1)
                    transp_chunk(3, 0)

            # bt2 wraps its missed k-tiles: bf16 kts then DR pairs
            # KTP0..JOIN_KTP-1; its s8 chain stops on the last wrap
            for kt in range(KT_F8):
                for bt_, nt_ in join:
                    mm(wps[bt_, nt_], bt_, nt_, kt, False, False)
            transp_chunk(3, 1)
            for ktp in range(KTP0, JOIN_KTP):
                last = ktp == JOIN_KTP - 1
                mm_dr(wps[2, 0], 2, 0, ktp, last)
                mm_s8(2, ktp, False, last)
                mm_dr(wps[2, 1], 2, 1, ktp, last)
            cast_chunk(3, 2)
            cast_chunk(3, 3)
            transp_chunk(3, 2)
            transp_chunk(3, 3)
            sx_reduce(3, 0)
            sx_reduce(3, 1)
            for bt_ in (0, 1, 2):
                compute_c(bt_)
            for g in wave + join:
                evict_out(wps[g], g[0], g[1])
        else:
            for bt_ in (2, 3):
                for g in range(4):
                    cast_T(bt_, g)

        # steady state: tranche-0 nt-paired groups for bt3..7, each pair
        # carrying the next bt's cast/transpose chain + sx reduces
        for bt in range(3, BT):
            group_pair(bt, (0, 1), prep=bt + 1 if bt + 1 < BT else None)

        # late phase: tranche-2/3 nt-paired groups; the last bt runs as
        # two split single groups so only one short chain trails the
        # final matmul
        for bt in range(BT - 1):
            group_pair(bt, (2, 3))
        group(BT - 1, 2, split=2)
        group(BT - 1, 3, split=2)

    with tile.TileContext(nc) as tc:
        with (
            tc.tile_pool(name="xraw", bufs=6) as xraw_pool,
            tc.tile_pool(name="xbf", bufs=2) as xbf_pool,
            tc.tile_pool(name="xT", bufs=1) as xT_pool,
            tc.tile_pool(name="wraw", bufs=8) as wraw_pool,
            tc.tile_pool(name="wbin", bufs=1) as wbin_pool,
            tc.tile_pool(name="osb", bufs=24) as out_pool,
            tc.tile_pool(name="ps", bufs=6, space="PSUM") as psum_pool,
            tc.tile_pool(name="tps", bufs=1, space="PSUM") as tpsum_pool,
            tc.tile_pool(name="pss", bufs=1, space="PSUM") as pss_pool,
            tc.tile_pool(name="corr", bufs=10) as corr_pool,
            tc.tile_pool(name="const", bufs=1) as const_pool,
        ):
            pools = (xraw_pool, xbf_pool, xT_pool, wraw_pool, wbin_pool,
                     out_pool, psum_pool, tpsum_pool, pss_pool, corr_pool,
                     const_pool)
            if repeat == 1:
                body(tc, pools)
            else:
                with tc.For_i(0, repeat, 1):
                    body(tc, pools)
    _dedup_ldweights(nc)
    nc.compile()
    return nc


def _ldw_key(ins):
    ap = ins.ins[0]
    bap = getattr(ap, "bass_ap", None)
    return (getattr(ap, "memref", None), getattr(bap, "offset", None),
            str(getattr(bap, "ap", None)), getattr(ins, "is_transpose", None))


def _dedup_ldweights(nc):
    """Remove PE weight reloads of the already-loaded stationary operand."""
    removed = 0
    for bb in nc.main_func.blocks:
        il = bb.instructions
        last_key = None
        drop = []
        for idx, ins in enumerate(il):
            if not isinstance(ins, mybir.InstLdweights):
                continue
            si = ins.sync_info
            has_sync = si is not None and (
                (si.on_wait and len(si.on_wait) > 0)
                or (si.on_update and len(si.on_update) > 0))
            key = _ldw_key(ins)
            if key == last_key and not has_sync:
                drop.append(idx)
                removed += 1
            else:
                last_key = key
        for idx in reversed(drop):
            del il[idx]
    return removed


_NC_CACHE = None


def _get_nc():
    global _NC_CACHE
    if _NC_CACHE is None:
        _NC_CACHE = build_kernel()
    return _NC_CACHE


def kernel(x: np.ndarray, weight: np.ndarray):
    assert x.shape == (B, K) and weight.shape == (K, N)
    x = np.ascontiguousarray(x, dtype=np.float32)
    weight = np.ascontiguousarray(weight, dtype=np.float32)
    nc = _get_nc()
    in_maps = [
        {"x": x[i * MB:(i + 1) * MB], "w": weight}
        for i in range(N_CORES)
    ]
    res = run_bass_kernel_spmd(nc, in_maps, core_ids=list(range(N_CORES)))
    return np.concatenate([res.results[i]["out"] for i in range(N_CORES)], axis=0)


# revision 21
# speedup vs baseline: 1.1134x; 1.1134x over previous
"""BinaryLinear kernel for 8x TRN2 NeuronCores.

out = x @ (weight > 0)  with x [8192, 2048] f32, weight [2048, 2048] f32.

Sharding: data-parallel over batch (1024 rows/core), weight replicated.

Per core (M=1024, K=2048, N=2048). The kernel is DMA-wire-bound: 32MiB
of DMA per core (8 x + 16 w in, 8 out) = 93.2us at the 360GB/s per-core
DMA rate. v2 (TimelineSim 97.3us, from the v1 baseline's 109.0us /
108us HW) pushes the PE work far enough below that floor (PE ~49%
busy) that the schedule hugs the wire: the DMA device runs back-to-back
from the ~2.0us launch latency to the ~1.5us epilogue with <0.4us of
mid-stream idle, inputs first and all 8MB of outputs streaming behind
them.

- Split-K mixed precision with MEAN-CORRECTION: k-tiles 0..3 run in
  bf16; k-tiles 4..15 run in fp8e4 DoubleRow (2 k-tiles per matmul,
  2x PE rate; binarized {0,1} weights are exact in fp8, x is fp8e4-
  quantized). The fp8 quantization error e_k = x8-x enters the output
  as sum_k e_k w_kn; writing w = 0.5 + (w-0.5), the 0.5*sum_k(e_k)
  term (half the error variance) is removed exactly:
    * S8[m] = sum_fp8k x8[k,m] accumulates via tiny piggyback DoubleRow
      matmuls (rhs = a [P,2,1] const-0.5 column) that share the
      stationary xT8 operand with the main matmuls (LDWEIGHTS dedup
      makes them ~free). All bts' accumulators live in one persistent
      PSUM bank (column per bt), memset once: a matmul with start=True
      zeroes the WHOLE bank, so every piggyback accumulates start=False.
    * Sx[m] = exact f32 row-sum of x over the fp8 k-range via DVE
      tensor_reduce on the raw x tiles;
    * c[m] = 0.5*(Sx - S8) is folded into every PSUM eviction as a
      per-partition tensor_scalar add on DVE.
  12 corrected fp8 k-tiles give the same 1.66e-2 rel err as the v1
  baseline's 6 uncorrected ones (HW-measured; gate 2e-2), and cut the
  PE matmul work from 89us to 51us - far under the DMA floor, so the
  whole late phase (nt2/nt3 groups gated on the last 23us of input)
  comfortably runs as plain serial kt-inner pairs.
- Weight streams in three column tranches (1024 + 512 + 512 cols),
  k-tile-major within each; every tranche-0 k-tile streams as two
  256KB nt-half DMAs so each arriving half immediately unlocks wave
  matmuls. DVE binarizes to {0,1} (bf16 or fp8) per 512-col quarter.
- Ramp: while tranche-0 k-tiles arrive, matmuls run kt/ktp-OUTER
  across 6 live banks (bt0,bt1,bt2)x(nt0,nt1) with bt2 joining at
  JOIN_KTP and wrapping its missed k-tiles afterwards; x transposes
  fill PE slack. The first four x/w DMAs are emitted before the
  ident/const setup so the wire starts immediately.
- Steady state: nt-paired 2-bank groups, kt-inner, consecutive
  matmuls sharing the stationary xT tile; each (0,1) pair carries the
  next bt's cast(DVE)/transpose(PE)/evict(ACT) chain interleaved; the
  last bt's nt2/nt3 run as half-width split groups so eviction
  overlaps the final matmuls.
- All out-DMAs ride the SP HWDGE ring EMITTED AFTER every input DMA:
  ring FIFO order gives inputs strict wire priority; a deep SBUF
  staging pool (24) absorbs evictions until the input stream drains.
  xraw/wraw pools are deep enough (6/8) that input-buffer recycling
  never stalls the wire on DVE backlog.
- x transposed 128x128-blockwise on the PE (is_transpose), 4 blocks
  per PSUM staging tile, contiguous segmented ACT eviction into per-bt
  xT (bf16) / xT8 (fp8) tiles; the first two startup chunks transpose
  raw f32 directly (their fp8 k-tiles single-round f32->fp8; all
  others double-round f32->bf16->fp8).
"""

import numpy as np

import concourse.bass as bass
import concourse.mybir as mybir
import concourse.tile as tile
from concourse import bacc
from concourse.bass_utils import run_bass_kernel_spmd
from concourse.masks import make_identity

B, K, N = 8192, 2048, 2048
N_CORES = 8
MB = B // N_CORES          # 1024 batch rows per core
P = 128
KT = K // P                # 16 k-tiles
BT = MB // P               # 8 batch tiles per core
NT = 4                     # output column blocks of 512
NB = N // NT               # 512
HW = K // 2                # 1024

F32 = mybir.dt.float32
BF16 = mybir.dt.bfloat16
F8 = mybir.dt.float8e4

# k-tiles >= KT_F8 run in fp8e4 DoubleRow; mean-corrected (see module
# docstring). KT_F8=4 -> 12 fp8 k-tiles, 6 DR pairs, rel err 1.66e-2.
KT_F8 = 4
KTP0 = KT_F8 // 2          # first DR pair index
NKTP = KT // 2 - KTP0      # number of DR pairs
JOIN_KTP = KTP0 + 2        # DR pair at which bt2 joins the ramp wave


def build_kernel(repeat: int = 1, mode: str = "full"):
    nc = bacc.Bacc(None, target_bir_lowering=False)
    x = nc.dram_tensor("x", [MB, K], F32, kind="ExternalInput")
    w = nc.dram_tensor("w", [K, N], F32, kind="ExternalInput")
    out = nc.dram_tensor("out", [MB, N], F32, kind="ExternalOutput")

    w3 = w[:].rearrange("(kt p) n -> p kt n", p=P)   # [128, 16, 2048]

    do_x = mode in ("full", "nomm", "xonly")
    do_w = mode in ("full", "nomm", "wonly")
    do_mm = mode in ("full", "mmonly")

    def body(tc, pools):
        (xraw_pool, xbf_pool, xT_pool, wraw_pool, wbin_pool,
         out_pool, psum_pool, tpsum_pool, pss_pool, corr_pool,
         const_pool) = pools

        ident = const_pool.tile([P, P], BF16, tag="ident", name="ident")
        ident32 = const_pool.tile([P, P], F32, tag="ident32", name="ident32")
        halfcol = const_pool.tile([P, 2], F8, tag="halfcol", name="halfcol")
        # one persistent PSUM bank holds every bt's 0.5*S8 accumulator
        # (column bt). A matmul with start=True zeroes the whole bank
        # (not just its own column), so the bank is memset once and every
        # piggyback matmul accumulates with start=False.
        pss_all = pss_pool.tile([P, BT], F32, tag="pss", name="pss")

        def const_setup():
            make_identity(nc, ident)
            make_identity(nc, ident32)
            nc.any.memset(halfcol[:], 0.5)
            nc.any.memset(pss_all[:], 0.0)

        xraw = {}   # (bt, half) -> [P, HW] f32
        xT = {}     # bt -> [P, KT_F8*P] bf16
        xT8 = {}    # bt -> [P, (KT-KT_F8)*P] fp8e4 (col = (kt-KT_F8)*P + m)
        wbin = {}   # (kt, nt) -> [P, NB] bf16      (kts < KT_F8)
        wbin8 = {}  # (ktp, nt) -> [P, 2*NB] fp8e4  (ktp in KTP0..KT//2-1)
        sxp = {}    # (bt, half) -> [P, 1] f32 partial exact row-sums
        pss = {}    # bt -> [P, 1] f32 psum tile accumulating 0.5*S8
        cbias = {}  # bt -> [P, 1] f32 sbuf correction tile
        evict_flip = [0]

        def _binarize(dst, src):
            nc.vector.tensor_scalar(out=dst, in0=src, scalar1=0.0,
                                    scalar2=None, op0=mybir.AluOpType.is_gt)

        def _wbin_dst(kt, nt):
            """Destination slice for a binarized [P, NB] w quarter."""
            if kt < KT_F8:
                wbin[kt, nt] = wbin_pool.tile(
                    [P, NB], BF16, tag=f"wbin_{kt}_{nt}", name=f"wb{kt}_{nt}")
                return wbin[kt, nt][:]
            ktp, j = divmod(kt, 2)
            if (ktp, nt) not in wbin8:
                wbin8[ktp, nt] = wbin_pool.tile(
                    [P, 2 * NB], F8, tag=f"wbin8_{ktp}_{nt}",
                    name=f"wb8{ktp}_{nt}")
            return wbin8[ktp, nt][:, j * NB:(j + 1) * NB]

        def dma_x_piece(bt, half, piece):
            if not do_x:
                return
            if piece == 0:
                xraw[bt, half] = xraw_pool.tile(
                    [P, HW], F32, tag=f"xraw_{half}", name=f"xr{bt}_{half}")
            t = xraw[bt, half]
            h0 = half * HW
            c = piece * NB
            nc.sync.dma_start(t[:, c:c + NB],
                              x[bt * P:(bt + 1) * P, h0 + c:h0 + c + NB])

        def dma_x(bt, half, quarters=False):
            if not do_x:
                return
            t = xraw_pool.tile([P, HW], F32, tag=f"xraw_{half}",
                               name=f"xr{bt}_{half}")
            h0 = half * HW
            if quarters:
                nc.sync.dma_start(t[:, :NB], x[bt * P:(bt + 1) * P,
                                               h0:h0 + NB])
                nc.sync.dma_start(t[:, NB:], x[bt * P:(bt + 1) * P,
                                               h0 + NB:h0 + HW])
            else:
                nc.sync.dma_start(t[:], x[bt * P:(bt + 1) * P, h0:h0 + HW])
            xraw[bt, half] = t

        def dma_w2(kt):
            """Tranche-0: [128,1024] w k-tile -> binarized quarters nt0,nt1."""
            dsts = [_wbin_dst(kt, j) for j in range(2)]
            if do_w:
                wr = wraw_pool.tile([P, HW], F32, tag="wraw2", name="wr")
                nc.sync.dma_start(wr[:], w3[:, kt, 0:HW])
                for j in range(2):
                    _binarize(dsts[j], wr[:, j * NB:(j + 1) * NB])
            else:
                for j in range(2):
                    nc.any.memset(dsts[j], 1.0)

        def dma_w1(kt, nt):
            """Tranche-2/3: [128,512] w k-tile quarter."""
            dst = _wbin_dst(kt, nt)
            if do_w:
                wr = wraw_pool.tile([P, NB], F32, tag="wraw1", name="wr")
                nc.sync.dma_start(wr[:], w3[:, kt, nt * NB:(nt + 1) * NB])
                _binarize(dst, wr[:])
            else:
                nc.any.memset(dst, 1.0)

        # ---- exact f32 row-sums of x over the fp8 k-range (DVE) ----
        def sx_reduce(bt, half):
            if not (do_x and do_mm):
                return
            lo = max(KT_F8 * P - half * HW, 0)
            t = corr_pool.tile([P, 1], F32, tag=f"sxp_{half}",
                               name=f"sx{bt}_{half}")
            nc.vector.tensor_reduce(
                t[:], xraw[bt, half][:, lo:HW],
                axis=mybir.AxisListType.X, op=mybir.AluOpType.add)
            sxp[bt, half] = t

        def compute_c(bt):
            """c[bt] = 0.5*(Sx - S8); the per-partition eviction bias."""
            if not do_mm:
                return
            sx = corr_pool.tile([P, 1], F32, tag="sx", name=f"sxc{bt}")
            if (bt, 0) in sxp:
                # sx = 0.5 * (sx_half0 + sx_half1)
                nc.vector.tensor_scalar(
                    out=sx[:], in0=sxp.pop((bt, 0))[:],
                    scalar1=sxp.pop((bt, 1))[:], scalar2=0.5,
                    op0=mybir.AluOpType.add, op1=mybir.AluOpType.mult)
            else:
                nc.any.memset(sx[:], 0.0)
            c = corr_pool.tile([P, 1], F32, tag="c", name=f"c{bt}")
            # pss holds 0.5*S8; c = 0.5*Sx - 0.5*S8 = (pss - sx) * -1
            nc.vector.tensor_scalar(
                out=c[:], in0=pss[bt], scalar1=sx[:], scalar2=-1.0,
                op0=mybir.AluOpType.subtract, op1=mybir.AluOpType.mult)
            cbias[bt] = c

        xbf = {}

        def cast_chunk(bt, ktg, on_dve=False):
            """Cast 512 cols of x(bt) f32 -> bf16 (ACT, or DVE for the
            steady-phase bts)."""
            if bt not in xT:
                xT[bt] = xT_pool.tile([P, KT_F8 * P], BF16, tag=f"xT_{bt}",
                                      name=f"xT_{bt}")
                xT8[bt] = xT_pool.tile([P, (KT - KT_F8) * P], F8,
                                       tag=f"xT8_{bt}", name=f"xT8_{bt}")
            if not do_x:
                if ktg == 0:
                    nc.any.memset(xT[bt][:], 1.0)
                    nc.any.memset(xT8[bt][:], 1.0)
                return
            half, off = divmod(ktg * 4 * P, HW)
            xb = xbf_pool.tile([P, 4 * P], BF16, tag=f"xbf_{ktg % 2}",
                               name=f"xbf{bt}_{ktg}")
            src = xraw[bt, half][:, off:off + 4 * P]
            if on_dve:
                nc.vector.tensor_copy(xb[:], src)
            else:
                nc.scalar.activation(xb[:], src,
                                     mybir.ActivationFunctionType.Copy)
            xbf[bt, ktg] = xb

        def _evict_T(dst_seg_f32, tp, bt, ktg):
            """Segmented eviction of a 4-block transposed chunk into
            xT (bf16 kts) and xT8 (fp8 kts)."""
            k0 = ktg * 4
            nbf = min(max(KT_F8 - k0, 0), 4)
            if nbf:
                nc.scalar.activation(
                    xT[bt][:, k0 * P:(k0 + nbf) * P], tp[:, :nbf * P],
                    mybir.ActivationFunctionType.Copy)
            if nbf < 4:
                f0 = (k0 + nbf - KT_F8) * P
                nc.scalar.activation(
                    xT8[bt][:, f0:f0 + (4 - nbf) * P], tp[:, nbf * P:],
                    mybir.ActivationFunctionType.Copy)

        def transp_f32_chunk(bt, ktg):
            """Startup only: transpose 4 blocks straight from raw f32 x
            (skips the cast on the critical path); PSUM staging borrows a
            main-pool bank; ACT eviction converts f32 -> bf16/fp8."""
            if bt not in xT:
                xT[bt] = xT_pool.tile([P, KT_F8 * P], BF16, tag=f"xT_{bt}",
                                      name=f"xT_{bt}")
                xT8[bt] = xT_pool.tile([P, (KT - KT_F8) * P], F8,
                                       tag=f"xT8_{bt}", name=f"xT8_{bt}")
            if not do_x:
                if ktg == 0:
                    nc.any.memset(xT[bt][:], 1.0)
                    nc.any.memset(xT8[bt][:], 1.0)
                return
            half, off = divmod(ktg * 4 * P, HW)
            tp = psum_pool.tile([P, 4 * P], F32, tag="ps", name="tpf")
            for i in range(4):
                nc.tensor.transpose(
                    tp[:, i * P:(i + 1) * P],
                    xraw[bt, half][:, off + i * P:off + (i + 1) * P],
                    ident32[:])
            _evict_T(None, tp, bt, ktg)

        def transp_chunk(bt, ktg):
            """PE: transpose 4 blocks into one PSUM staging tile; ACT
            evicts contiguously into xT[bt] / xT8[bt]."""
            if not do_x:
                return
            xb = xbf.pop((bt, ktg))
            tp = tpsum_pool.tile([P, 4 * P], BF16, tag="tps", name="tp")
            for i in range(4):
                nc.tensor.transpose(tp[:, i * P:(i + 1) * P],
                                    xb[:, i * P:(i + 1) * P], ident[:])
            _evict_T(None, tp, bt, ktg)

        def cast_T(bt, ktg):
            cast_chunk(bt, ktg)
            transp_chunk(bt, ktg)

        def mm(ps, bt, nt, kt, start, stop):
            nc.tensor.matmul(
                ps[:], xT[bt][:, kt * P:(kt + 1) * P], wbin[kt, nt][:],
                start=start, stop=stop)

        def mm_dr(ps, bt, nt, ktp, stop, start=False):
            """fp8 DoubleRow matmul: contracts k-tiles 2*ktp and 2*ktp+1
            in one pass (2 fp8 weights per PE cell, 2x rate)."""
            a = (ktp - KTP0) * 2 * P
            lhsT = xT8[bt][:, a:a + 2 * P].rearrange("p (j m) -> p j m", j=2)
            rhs = wbin8[ktp, nt][:].rearrange("p (j n) -> p j n", j=2)
            nc.tensor.matmul(ps[:], lhsT, rhs, start=start, stop=stop,
                             perf_mode=mybir.MatmulPerfMode.DoubleRow)

        def mm_s8(bt, ktp, start, stop):
            """Piggyback DR matmul accumulating 0.5*S8[m] into column bt
            of the shared pss bank. Shares the stationary xT8 pair with
            the main matmul emitted just before it, so its LDWEIGHTS
            dedups away."""
            if not do_mm:
                return
            pss[bt] = pss_all[:, bt:bt + 1]
            a = (ktp - KTP0) * 2 * P
            lhsT = xT8[bt][:, a:a + 2 * P].rearrange("p (j m) -> p j m", j=2)
            rhs = halfcol[:].rearrange("p (j n) -> p j n", j=2)
            nc.tensor.matmul(pss[bt], lhsT, rhs, start=False, stop=stop,
                             perf_mode=mybir.MatmulPerfMode.DoubleRow,
                             skip_group_check=True)

        def evict_out(ps, bt, nt, off=0, ring=None):
            """PSUM -> SBUF staging with the mean-correction bias folded
            in (DVE tensor_scalar add of c[bt]) -> DRAM via the SP HWDGE
            ring (emitted after every input DMA: FIFO order gives inputs
            strict wire priority). Evicts the whole ps tile to out cols
            [nt*NB + off, ...)."""
            ring = ring or nc.sync
            wd = ps.shape[1]
            ot = out_pool.tile([P, wd], F32, tag="osb", name="ot")
            if do_mm:
                nc.vector.tensor_scalar(
                    out=ot[:], in0=ps[:],
                    scalar1=cbias[bt][:], scalar2=None,
                    op0=mybir.AluOpType.add)
            else:
                nc.vector.tensor_copy(ot[:], ps[:])
            ring.dma_start(
                out[bt * P:(bt + 1) * P,
                    nt * NB + off:nt * NB + off + wd], ot[:])

        def group(bt, nt, ring=None, split=1):
            """Single-nt group; split=2 runs two half-width (256-col)
            accumulations so the first half's eviction + out-DMA overlap
            the second half's matmuls (shrinks the kernel tail)."""
            ring = ring or nc.sync
            wd = NB // split
            for h in range(split):
                ps = psum_pool.tile([P, wd], F32, tag="ps", name="ps")
                if do_mm:
                    for kt in range(KT_F8):
                        nc.tensor.matmul(
                            ps[:], xT[bt][:, kt * P:(kt + 1) * P],
                            wbin[kt, nt][:, h * wd:(h + 1) * wd],
                            start=(kt == 0), stop=False)
                    for ktp in range(KTP0, KT // 2):
                        a = (ktp - KTP0) * 2 * P
                        lhsT = xT8[bt][:, a:a + 2 * P].rearrange(
                            "p (j m) -> p j m", j=2)
                        rhs = wbin8[ktp, nt][:].rearrange(
                            "p (j n) -> p j n", j=2)[:, :, h * wd:(h + 1) * wd]
                        nc.tensor.matmul(
                            ps[:], lhsT, rhs, start=False,
                            stop=(ktp == KT // 2 - 1),
                            perf_mode=mybir.MatmulPerfMode.DoubleRow)
                else:
                    nc.any.memset(ps[:], 0.0)
                evict_out(ps, bt, nt, off=h * wd, ring=ring)

        def group_pair(bt, nts, ring=None, prep=None):
            """Two nt blocks of one bt, kt-inner with consecutive matmuls
            sharing the stationary xT tile. The (nt0, nt1) pair carries
            the bt's S8 piggyback chain; `prep` names a bt whose
            cast/transpose chain is emitted interleaved with the DR
            pairs."""
            s8_here = nts[0] == 0
            psa = psum_pool.tile([P, NB], F32, tag="ps", name="ps")
            psb = psum_pool.tile([P, NB], F32, tag="ps", name="ps")
            prep_at = {2: 0, 3: 1, 4: 2, 5: 3}
            if do_mm:
                for kt in range(KT_F8):
                    mm(psa, bt, nts[0], kt, kt == 0, False)
                    mm(psb, bt, nts[1], kt, kt == 0, False)
                for ktp in range(KTP0, KT // 2):
                    last = ktp == KT // 2 - 1
                    mm_dr(psa, bt, nts[0], ktp, last)
                    if s8_here:
                        mm_s8(bt, ktp, ktp == KTP0, last)
                    mm_dr(psb, bt, nts[1], ktp, last)
                    if prep is not None and ktp in prep_at:
                        g = prep_at[ktp]
                        cast_chunk(prep, g, on_dve=True)
                        transp_chunk(prep, g)
                        if g == 3:
                            sx_reduce(prep, 0)
                            sx_reduce(prep, 1)
            else:
                nc.any.memset(psa[:], 0.0)
                nc.any.memset(psb[:], 0.0)
                if prep is not None:
                    for g in range(4):
                        cast_T(prep, g)
            if s8_here:
                compute_c(bt)
            evict_out(psa, bt, nts[0], ring=ring)
            evict_out(psb, bt, nts[1], ring=ring)

        # ================= emission script =================
        # startup: first x chunks and w k-tiles interleaved so the first
        # wave matmul can issue as early as possible
        dma_x(0, 0, quarters=True)
        dma_w1(0, 0)
        dma_w1(0, 1)
        dma_x(0, 1)
        const_setup()
        dma_w1(1, 0)
        dma_x_piece(1, 0, 0)
        dma_w1(1, 1)
        dma_x_piece(1, 0, 1)
        dma_w1(2, 0)
        dma_x_piece(1, 1, 0)
        dma_w1(2, 1)
        dma_x_piece(1, 1, 1)
        dma_w1(3, 0)
        dma_x_piece(2, 0, 0)
        dma_w1(3, 1)
        dma_x_piece(2, 0, 1)
        dma_w1(4, 0)
        dma_x_piece(2, 1, 0)
        dma_w1(4, 1)
        dma_x_piece(2, 1, 1)
        for kt in range(5, KT):
            dma_w1(kt, 0)
            dma_w1(kt, 1)
            if kt == 7:
                dma_x(3, 0)
        dma_x(3, 1)
        # rest of x right after tranche-0; then tranches 2 and 3
        for bt in range(4, BT):
            dma_x(bt, 0)
            dma_x(bt, 1)
        for kt in range(KT):
            dma_w1(kt, 2)
        for kt in range(KT):
            dma_w1(kt, 3)

        # transposes for bt0, bt1 up front (bt0's first two chunks
        # transpose raw f32 directly - no cast latency on the critical
        # startup path)
        transp_f32_chunk(0, 0)
        transp_f32_chunk(0, 1)
        cast_T(0, 2)
        cast_T(0, 3)
        for ktg in range(4):
            cast_T(1, ktg)
        sx_reduce(0, 0)
        sx_reduce(0, 1)
        sx_reduce(1, 0)
        sx_reduce(1, 1)

        # ramp wave: ktp-outer, (bt0,bt1)x(nt0,nt1); bt2 joins at
        # JOIN_KTP and wraps around; transposes for bt2 fill PE slack,
        # bt3's h0 casts are prefetched near the wave end
        if do_mm:
            wave = [(0, 0), (0, 1), (1, 0), (1, 1)]
            wps = {g: psum_pool.tile([P, NB], F32, tag="ps", name=f"wps{g}")
                   for g in wave}
            join = [(2, 0), (2, 1)]
            for kt in range(KT_F8):
                for bt_, nt_ in wave:
                    mm(wps[bt_, nt_], bt_, nt_, kt, kt == 0, False)
            for ktp in range(KTP0, KT // 2):
                last = ktp == KT // 2 - 1
                mm_dr(wps[0, 0], 0, 0, ktp, last)
                mm_s8(0, ktp, ktp == KTP0, last)
                mm_dr(wps[0, 1], 0, 1, ktp, last)
                mm_dr(wps[1, 0], 1, 0, ktp, last)
                mm_s8(1, ktp, ktp == KTP0, last)
                mm_dr(wps[1, 1], 1, 1, ktp, last)
                if ktp == JOIN_KTP - 2:
                    cast_T(2, 0)
                    cast_T(2, 1)
                elif ktp == JOIN_KTP - 1:
                    cast_T(2, 2)
                    cast_T(2, 3)
                    sx_reduce(2, 0)
                    sx_reduce(2, 1)
                    for g in join:
                        wps[g] = psum_pool.tile([P, NB], F32, tag="ps",
                                                name=f"wps{g}")
                if ktp >= JOIN_KTP:
                    mm_dr(wps[2, 0], 2, 0, ktp, False, start=ktp == JOIN_KTP)
                    mm_s8(2, ktp, ktp == JOIN_KTP, False)
                    mm_dr(wps[2, 1], 2, 1, ktp, False)
                if ktp == KT // 2 - 2:
                    cast_chunk(3, 0)
                elif ktp == KT // 2 - 1:
                    cast_chunk(3, 1)
                    transp_chunk(3, 0)

            # bt2 wraps its missed k-tiles: bf16 kts then DR pairs
            # KTP0..JOIN_KTP-1; its s8 chain stops on the last wrap
            for kt in range(KT_F8):
                for bt_, nt_ in join:
                    mm(wps[bt_, nt_], bt_, nt_, kt, False, False)
            transp_chunk(3, 1)
            for ktp in range(KTP0, JOIN_KTP):
                last = ktp == JOIN_KTP - 1
                mm_dr(wps[2, 0], 2, 0, ktp, last)
                mm_s8(2, ktp, False, last)
                mm_dr(wps[2, 1], 2, 1, ktp, last)
            cast_chunk(3, 2)
            cast_chunk(3, 3)
            transp_chunk(3, 2)
            transp_chunk(3, 3)
            sx_reduce(3, 0)
            sx_reduce(3, 1)
            for bt_ in (0, 1, 2):
                compute_c(bt_)
            for g in wave + join:
                evict_out(wps[g], g[0], g[1])
        else:
            for bt_ in (2, 3):
                for g in range(4):
                    cast_T(bt_, g)

        # steady state: tranche-0 nt-paired groups for bt3..7, each pair
        # carrying the next bt's cast/transpose chain + sx reduces
        for bt in range(3, BT):
            group_pair(bt, (0, 1), prep=bt + 1 if bt + 1 < BT else None)

        # late phase: tranche-2/3 nt-paired groups; the last bt runs as
        # two split single groups so only one short chain trails the
        # final matmul
        for bt in range(BT - 1):
            group_pair(bt, (2, 3))
        group(BT - 1, 2, split=2)
        group(BT - 1, 3, split=2)

    with tile.TileContext(nc) as tc:
        with (
            tc.tile_pool(name="xraw", bufs=6) as xraw_pool,
            tc.tile_pool(name="xbf", bufs=2) as xbf_pool,
            tc.tile_pool(name="xT", bufs=1) as xT_pool,
            tc.tile_pool(name="wraw", bufs=8) as wraw_pool,
            tc.tile_pool(name="wbin", bufs=1) as wbin_pool,
            tc.tile_pool(name="osb", bufs=24) as out_pool,
            tc.tile_pool(name="ps", bufs=6, space="PSUM") as psum_pool,
            tc.tile_pool(name="tps", bufs=1, space="PSUM") as tpsum_pool,
            tc.tile_pool(name="pss", bufs=1, space="PSUM") as pss_pool,
            tc.tile_pool(name="corr", bufs=10) as corr_pool,
            tc.tile_pool(name="const", bufs=1) as const_pool,
        ):
            pools = (xraw_pool, xbf_pool, xT_pool, wraw_pool, wbin_pool,
                     out_pool, psum_pool, tpsum_pool, pss_pool, corr_pool,
                     const_pool)
            if repeat == 1:
                body(tc, pools)
            else:
                with tc.For_i(0, repeat, 1):
                    body(tc, pools)
    _dedup_ldweights(nc)
    nc.compile()
    return nc


def _ldw_key(ins):
    ap = ins.ins[0]
    bap = getattr(ap, "bass_ap", None)
    return (getattr(ap, "memref", None), getattr(bap, "offset", None),
            str(getattr(bap, "ap", None)), getattr(ins, "is_transpose", None))


def _dedup_ldweights(nc):
    """Remove PE weight reloads of the already-loaded stationary operand."""
    removed = 0
    for bb in nc.main_func.blocks:
        il = bb.instructions
        last_key = None
        drop = []
        for idx, ins in enumerate(il):
            if not isinstance(ins, mybir.InstLdweights):
                continue
            si = ins.sync_info
            has_sync = si is not None and (
                (si.on_wait and len(si.on_wait) > 0)
                or (si.on_update and len(si.on_update) > 0))
            key = _ldw_key(ins)
            if key == last_key and not has_sync:
                drop.append(idx)
                removed += 1
            else:
                last_key = key
        for idx in reversed(drop):
            del il[idx]
    return removed


_NC_CACHE = None


def _get_nc():
    global _NC_CACHE
    if _NC_CACHE is None:
        _NC_CACHE = build_kernel()
    return _NC_CACHE


def kernel(x: np.ndarray, weight: np.ndarray):
    assert x.shape == (B, K) and weight.shape == (K, N)
    x = np.ascontiguousarray(x, dtype=np.float32)
    weight = np.ascontiguousarray(weight, dtype=np.float32)
    nc = _get_nc()
    in_maps = [
        {"x": x[i * MB:(i + 1) * MB], "w": weight}
        for i in range(N_CORES)
    ]
    res = run_bass_kernel_spmd(nc, in_maps, core_ids=list(range(N_CORES)))
    return np.concatenate([res.results[i]["out"] for i in range(N_CORES)], axis=0)
